# revision 16
# baseline (speedup 1.0000x reference)
"""Trainium2 Bass kernel for additive (Bahdanau) attention GNN message passing.

score[n, m] = v . tanh(a[n] + b[m]),  a = x1 @ W1.T, b = x2 @ W2.T + bc
w = softmax(score, axis=n);  ctx[m] = w[:, m].T @ x1
out = tanh(concat([att, ctx_s, ctx_e]) @ W_lin.T + b_lin)

Sharding: attender dim M=1024 split across 8 cores (128 each); attendees and
params replicated. No collectives.

Key trick: the per-(n,m,h) tanh (25M ACT elems/core in the naive scheme) is
replaced by a separable harmonic expansion
    tanh(s) ~ sum_k alpha_k sin(k w0 s),   s = a + b
    sin(k w0 (a+b)) = sin(k w0 a) cos(k w0 b) + cos(k w0 a) sin(k w0 b)
so the O(N*M*H) work becomes PE matmuls contracting h for each harmonic,
and the nonlinearity cost drops to O((N+M)*H) basis evaluations.

The HW ACT Sin table is only valid for |arg| < ~pi, so only small-argument
sins run on ACT (.5*w0*x, w0*x, 1.5*w0*x; args <= 2.7 rad) plus Squares;
cosines come from cos(2t) = 1 - 2 sin^2(t) and higher harmonics from
bf16 Chebyshev product recurrences on DVE. Weighting v*alpha folds into the
small b-side tiles (gpsimd). Scores accumulate in PSUM over 2F matmuls per
128-attendee chunk; softmax sums fall out of the ctx matmul via a ones
column in the attendee image; final linear runs in f32r.

PSUM accumulation note: start=True clears the has_written bits of the WHOLE
bank, so only the first matmul touching each bank may set it; later
first-writes to other regions rely on per-element overwrite-then-accumulate.
"""

import numpy as np
from ml_dtypes import bfloat16

import concourse.bass as bass
import concourse.tile as tile
from concourse import bacc, masks, mybir
from concourse.bass_utils import run_bass_kernel_spmd

F32 = mybir.dt.float32
F32R = mybir.dt.float32r
BF16 = mybir.dt.bfloat16
AF = mybir.ActivationFunctionType
OP = mybir.AluOpType

H = 128      # hidden
A = 256      # attention (output) size
N_S = 1024   # attendee statements
N_E = 512    # attendee EREs
M = 1024     # attenders
NC = 8       # cores
ML = M // NC # attenders per core
NT = N_S + N_E  # 1536
NCH = NT // 128  # 12 chunks of attendees
CW = 129     # x-image chunk width: 128 attendee cols + a ones column

W0 = 0.267059
KS = (1, 2, 3, 4, 6)
ALPHA = (1.17663, 0.12087, 0.17747, 0.13768, 0.13409)

_CACHE = {}


def _build():
    nc = bacc.Bacc(
        "TRN2", target_bir_lowering=False, debug=False, num_devices=NC
    )

    d_x16 = nc.dram_tensor("x16", [128, NCH * CW], BF16, kind="ExternalInput").ap()
    d_stmtsT = nc.dram_tensor("stmtsT", [128, N_S], BF16, kind="ExternalInput").ap()
    d_eresT = nc.dram_tensor("eresT", [128, N_E], BF16, kind="ExternalInput").ap()
    d_wa = nc.dram_tensor("wa", [128, 4 * H + ML], BF16, kind="ExternalInput").ap()
    d_attTf = nc.dram_tensor("attTf", [128, ML], F32, kind="ExternalInput").ap()
    d_wlinT = nc.dram_tensor("wlinT", [128, 3 * A], F32, kind="ExternalInput").ap()
    d_vb = nc.dram_tensor("vb", [128, 4], F32, kind="ExternalInput").ap()
    d_blin = nc.dram_tensor("blin", [1, A], F32, kind="ExternalInput").ap()
    d_out = nc.dram_tensor("out", [ML, A], F32, kind="ExternalOutput").ap()

    with tile.TileContext(nc) as tc:
        _emit(nc, tc, d_x16, d_stmtsT, d_eresT, d_wa,
              d_attTf, d_wlinT, d_vb, d_blin, d_out)

    nc.compile()
    return nc


def _emit(nc, tc, d_x16, d_stmtsT, d_eresT, d_wa,
          d_attTf, d_wlinT, d_vb, d_blin, d_out):
    from contextlib import ExitStack

    ctx = ExitStack()
    with ctx:
        const = ctx.enter_context(tc.tile_pool(name="const", bufs=1))
        bpool = ctx.enter_context(tc.tile_pool(name="bpool", bufs=1))
        apool = ctx.enter_context(tc.tile_pool(name="apool", bufs=1))
        ps_a = ctx.enter_context(
            tc.tile_pool(name="ps_a", bufs=1, space=bass.MemorySpace.PSUM))
        ps_score = ctx.enter_context(
            tc.tile_pool(name="ps_score", bufs=1, space=bass.MemorySpace.PSUM))
        ps_small = ctx.enter_context(
            tc.tile_pool(name="ps_small", bufs=1, space=bass.MemorySpace.PSUM))

        # ---- gpsimd init + ACT table warm ----
        ident = const.tile([128, 128], F32)
        masks.make_identity(nc, ident[:])
        ones_row = const.tile([1, 128], F32)
        nc.gpsimd.memset(ones_row[:], 1.0)
        scratch = const.tile([128, 1], F32)
        nc.gpsimd.memset(scratch[:], 0.0)
        nc.scalar.activation(scratch[:], scratch[:], AF.Sin)  # load trig table

        # ---- DMAs: packed wide-row images; sync carries the critical path ----
        sb_wa = const.tile([128, 4 * H + ML], BF16)
        nc.sync.dma_start(sb_wa[:], d_wa[:, :])
        sb_wT = sb_wa[:, 0:4 * H]
        sb_attT16 = sb_wa[:, 4 * H:4 * H + ML]
        sb_stmtsT = const.tile([128, N_S], BF16)
        nc.sync.dma_start(sb_stmtsT[:], d_stmtsT[:, :])
        sb_eresT = const.tile([128, N_E], BF16)
        nc.sync.dma_start(sb_eresT[:], d_eresT[:, :])

        sb_vb = const.tile([128, 4], F32)
        nc.gpsimd.dma_start(sb_vb[:], d_vb[:, :])
        sb_x16 = const.tile([128, NCH * CW], BF16)
        nc.gpsimd.dma_start(sb_x16[:], d_x16[:, :])
        sb_attTf = const.tile([128, ML], F32R)
        nc.gpsimd.dma_start(sb_attTf[:], d_attTf[:, :])
        sb_wlinT = const.tile([128, 3 * A], F32R)
        nc.gpsimd.dma_start(sb_wlinT[:], d_wlinT[:, :])
        sb_blin = const.tile([1, A], F32)
        nc.gpsimd.dma_start(sb_blin[0:1, :], d_blin[0:1, :])

        # ---- front matmuls ----
        # bT for both sets -> one [128, 258] PSUM tile (tag shared with ctx)
        ps_bT = ps_small.tile([128, 2 * CW], F32, tag="ctx", name="ps_bT")
        nc.tensor.matmul(ps_bT[:, 0:ML], sb_wT[:, 128:256], sb_attT16,
                         start=True, stop=True)
        nc.tensor.matmul(ps_bT[:, CW:CW + ML], sb_wT[:, 384:512], sb_attT16,
                         start=True, stop=True)
        sb_b2 = const.tile([128, 2 * ML], F32)
        nc.vector.tensor_scalar_add(sb_b2[:, 0:ML], ps_bT[:, 0:ML],
                                    sb_vb[:, 2:3])
        nc.vector.tensor_scalar_add(sb_b2[:, ML:2 * ML], ps_bT[:, CW:CW + ML],
                                    sb_vb[:, 3:4])

        # aT for both sets -> one [128, 1536] PSUM tile (bank-aligned pieces)
        ps_aT = ps_a.tile([128, NT], F32, tag="aT", name="ps_aT")
        nc.tensor.matmul(ps_aT[:, 0:512], sb_wT[:, 0:128],
                         sb_stmtsT[:, 0:512], start=True, stop=True)
        nc.tensor.matmul(ps_aT[:, 512:1024], sb_wT[:, 0:128],
                         sb_stmtsT[:, 512:1024], start=True, stop=True)
        nc.tensor.matmul(ps_aT[:, 1024:1536], sb_wT[:, 256:384],
                         sb_eresT[:], start=True, stop=True)

        # att + b_lin parts of the final linear (f32r, off critical path)
        ps_out = ps_small.tile([128, A], F32, tag="out")
        nc.tensor.matmul(ps_out[:], sb_attTf[:], sb_wlinT[:, 0:A],
                         start=True, stop=False, skip_group_check=True)
        nc.tensor.matmul(ps_out[:], ones_row[0:1, :], sb_blin[0:1, :],
                         start=False, stop=False, skip_group_check=True)

        # ---- basis generation helpers ----
        def gen_basis(pool, src_ap, width, name, act_square_s3):
            """Emit sin/cos harmonic tiles of src (bf16, [128, width]).

            Returns dict k -> (sin_tile, cos_tile)."""
            t = {}

            def tl(nm):
                return pool.tile([128, width], BF16, name=f"{nm}_{name}")

            u1, s1, u3 = tl("u1"), tl("s1"), tl("u3")
            nc.scalar.activation(u1[:], src_ap, AF.Sin, scale=0.5 * W0)
            nc.scalar.activation(s1[:], src_ap, AF.Sin, scale=W0)
            nc.scalar.activation(u3[:], src_ap, AF.Sin, scale=1.5 * W0)
            squ1, squ3 = tl("squ1"), tl("squ3")
            nc.scalar.activation(squ1[:], u1[:], AF.Square)
            nc.scalar.activation(squ3[:], u3[:], AF.Square)
            c1, c3 = tl("c1"), tl("c3")
            nc.vector.tensor_scalar(c1[:], squ1[:], -2.0, 1.0, OP.mult, OP.add)
            c1two = tl("c1two")
            nc.vector.tensor_scalar_mul(c1two[:], c1[:], 2.0)
            nc.vector.tensor_scalar(c3[:], squ3[:], -2.0, 1.0, OP.mult, OP.add)
            s2 = tl("s2")
            nc.vector.tensor_tensor(s2[:], c1two[:], s1[:], OP.mult)
            c2t, c2 = tl("c2t"), tl("c2")
            nc.vector.tensor_tensor(c2t[:], c1two[:], c1[:], OP.mult)
            nc.vector.tensor_scalar_sub(c2[:], c2t[:], 1.0)
            c2two = tl("c2two")
            nc.vector.tensor_scalar_mul(c2two[:], c2[:], 2.0)
            s3t, s3 = tl("s3t"), tl("s3")
            nc.vector.tensor_tensor(s3t[:], c1two[:], s2[:], OP.mult)
            nc.vector.tensor_tensor(s3[:], s3t[:], s1[:], OP.subtract)
            s4 = tl("s4")
            nc.vector.tensor_tensor(s4[:], c2two[:], s2[:], OP.mult)
            c4t, c4 = tl("c4t"), tl("c4")
            nc.vector.tensor_tensor(c4t[:], c2two[:], c2[:], OP.mult)
            nc.vector.tensor_scalar_sub(c4[:], c4t[:], 1.0)
            s6t, s6 = tl("s6t"), tl("s6")
            nc.vector.tensor_tensor(s6t[:], c2two[:], s4[:], OP.mult)
            nc.vector.tensor_tensor(s6[:], s6t[:], s2[:], OP.subtract)
            sqs3, c6 = tl("sqs3"), tl("c6")
            if act_square_s3:
                nc.scalar.activation(sqs3[:], s3[:], AF.Square)
            else:
                nc.vector.tensor_tensor(sqs3[:], s3[:], s3[:], OP.mult)
            nc.vector.tensor_scalar(c6[:], sqs3[:], -2.0, 1.0, OP.mult, OP.add)
            t[1] = (s1, c1)
            t[2] = (s2, c2)
            t[3] = (s3, c3)
            t[4] = (s4, c4)
            t[6] = (s6, c6)
            return t

        # b-side basis (small, both sets side by side) — DVE squares for s3
        bt = gen_basis(bpool, sb_b2[:], 2 * ML, "b", act_square_s3=False)

        # b-side weighting: w = bf16(v_set * alpha_k * tile_half), on gpsimd
        wsin = {}
        wcos = {}
        for k, al in zip(KS, ALPHA):
            ws = bpool.tile([128, 2 * ML], BF16, name=f"wsin{k}")
            wc = bpool.tile([128, 2 * ML], BF16, name=f"wcos{k}")
            for half, vcol in ((0, 0), (1, 1)):
                lo = half * ML
                nc.gpsimd.tensor_scalar(ws[:, lo:lo + ML],
                                        bt[k][0][:, lo:lo + ML],
                                        sb_vb[:, vcol:vcol + 1], al,
                                        OP.mult, OP.mult)
                nc.gpsimd.tensor_scalar(wc[:, lo:lo + ML],
                                        bt[k][1][:, lo:lo + ML],
                                        sb_vb[:, vcol:vcol + 1], al,
                                        OP.mult, OP.mult)
            wsin[k] = ws
            wcos[k] = wc

        # a-side basis (big tiles) — ACT square for s3 (balances engines)
        at = gen_basis(apool, ps_aT[:], NT, "a", act_square_s3=True)

        # ---- score matmuls: ps_sT[n_l, (c, m)] += sum_k sa*wcb + ca*wsb ----
        # start=True clears the has_written bits of the WHOLE PSUM bank, so
        # only the first matmul touching each bank (chunks 0/4/8) may set it;
        # later first-writes to other chunk regions rely on the per-element
        # overwrite-then-accumulate semantics.
        ps_sT = ps_score.tile([128, NT], F32)
        for ki, k in enumerate(KS):
            sa, ca = at[k]
            for trig in (0, 1):
                src = sa if trig == 0 else ca
                rhs = wcos[k] if trig == 0 else wsin[k]
                for c in range(NCH):
                    half = 0 if c < 8 else 1
                    lo = half * ML
                    nc.tensor.matmul(
                        ps_sT[:, c * 128:(c + 1) * 128],
                        src[:, c * 128:(c + 1) * 128],
                        rhs[:, lo:lo + ML],
                        start=(ki == 0 and trig == 0 and c % 4 == 0),
                        stop=(ki == len(KS) - 1 and trig == 1 and c % 4 == 3),
                        skip_group_check=True)

        # ---- epilogue: softmax + ctx + final linear + store ----
        sb_E = bpool.tile([128, NT], BF16)
        nc.scalar.activation(sb_E[:, 0:N_S], ps_sT[:, 0:N_S], AF.Exp)
        nc.scalar.activation(sb_E[:, N_S:NT], ps_sT[:, N_S:NT], AF.Exp)

        ps_ctx = ps_small.tile([128, 2 * CW], F32, tag="ctx", name="ps_ctx")
        for c in range(8):
            nc.tensor.matmul(ps_ctx[:, 0:CW],
                             sb_E[:, c * 128:(c + 1) * 128],
                             sb_x16[:, c * CW:(c + 1) * CW],
                             start=(c == 0), stop=(c == 7))
        for c in range(8, 12):
            nc.tensor.matmul(ps_ctx[:, CW:2 * CW],
                             sb_E[:, c * 128:(c + 1) * 128],
                             sb_x16[:, c * CW:(c + 1) * CW],
                             start=(c == 8), stop=(c == 11))

        sb_recip = bpool.tile([128, 2], F32)
        nc.vector.reciprocal(sb_recip[:, 0:1], ps_ctx[:, H:H + 1])
        nc.vector.reciprocal(sb_recip[:, 1:2], ps_ctx[:, CW + H:CW + H + 1])
        sb_ctx = bpool.tile([128, 2 * H], F32)
        nc.vector.tensor_scalar_mul(sb_ctx[:, 0:H], ps_ctx[:, 0:H],
                                    sb_recip[:, 0:1])
        nc.vector.tensor_scalar_mul(sb_ctx[:, H:2 * H], ps_ctx[:, CW:CW + H],
                                    sb_recip[:, 1:2])

        # transpose ctx halves [m, h] -> [h, m] via PE, copy to f32r
        ps_tr = ps_a.tile([128, NT], F32, tag="aT", name="ps_tr")
        nc.tensor.matmul(ps_tr[:, 0:128], sb_ctx[:, 0:H], ident[:],
                         is_transpose=True)
        nc.tensor.matmul(ps_tr[:, 512:640], sb_ctx[:, H:2 * H], ident[:],
                         is_transpose=True)
        sb_ctxT = bpool.tile([128, 2 * H], F32R)
        nc.vector.tensor_copy(sb_ctxT[:, 0:H], ps_tr[:, 0:128])
        nc.vector.tensor_copy(sb_ctxT[:, H:2 * H], ps_tr[:, 512:640])

        nc.tensor.matmul(ps_out[:], sb_ctxT[:, 0:H], sb_wlinT[:, A:2 * A],
                         start=False, stop=False, skip_group_check=True)
        nc.tensor.matmul(ps_out[:], sb_ctxT[:, H:2 * H], sb_wlinT[:, 2 * A:3 * A],
                         start=False, stop=True, skip_group_check=True)

        sb_out = bpool.tile([128, A], F32)
        nc.scalar.activation(sb_out[:], ps_out[:], AF.Tanh)
        nc.sync.dma_start(d_out[:, :], sb_out[:])


def _get_nc():
    if "nc" not in _CACHE:
        _CACHE["nc"] = _build()
    return _CACHE["nc"]


def _prep_inputs(inputs):
    """Host-side layout prep: transposes / bf16 casts / SBUF-image packing."""
    f = {k: np.ascontiguousarray(np.asarray(v, np.float32))
         for k, v in inputs.items()}
    stmts, eres = f["attendee_stmts"], f["attendee_eres"]
    ws, we, wlin = f["Ws_concat"], f["We_concat"], f["W_lin"]

    # x image: chunk c holds attendees [c*128, (c+1)*128) as [n_local, h],
    # plus a trailing ones column (turns the ctx matmul into ctx|sum)
    x = np.empty((128, NCH * CW), np.float32)
    for c in range(8):
        x[:, c * CW:c * CW + H] = stmts[c * 128:(c + 1) * 128]
        x[:, c * CW + H] = 1.0
    for c in range(8, 12):
        x[:, c * CW:c * CW + H] = eres[(c - 8) * 128:(c - 7) * 128]
        x[:, c * CW + H] = 1.0
    vb = np.ascontiguousarray(
        np.stack([f["vs_single"], f["ve_single"], f["bs_concat"],
                  f["be_concat"]], axis=1))
    shared = {
        "x16": np.ascontiguousarray(x.astype(bfloat16)),
        "stmtsT": np.ascontiguousarray(stmts.T.astype(bfloat16)),
        "eresT": np.ascontiguousarray(eres.T.astype(bfloat16)),

        "wlinT": np.ascontiguousarray(np.concatenate(
            [wlin[:, 0:H].T, wlin[:, H:2 * H].T, wlin[:, 2 * H:3 * H].T],
            axis=1)),
        "vb": vb,
        "blin": np.ascontiguousarray(f["b_lin"][None, :]),
    }
    wT16 = np.concatenate(
        [ws[:, :H].T, ws[:, H:].T, we[:, :H].T, we[:, H:].T], axis=1)
    att = f["attender"]
    in_maps = []
    for i in range(NC):
        attT = np.ascontiguousarray(att[i * ML:(i + 1) * ML].T)
        wa = np.concatenate([wT16, attT], axis=1).astype(bfloat16)
        in_maps.append(dict(shared, attTf=attT,
                            wa=np.ascontiguousarray(wa)))
    return in_maps


def kernel(**inputs) -> np.ndarray:
    nc = _get_nc()
    in_maps = _prep_inputs(inputs)
    res = run_bass_kernel_spmd(nc, in_maps, list(range(NC)))
    return np.concatenate(
        [res.results[i]["out"].astype(np.float32) for i in range(NC)], axis=0)


# revision 23
# speedup vs baseline: 1.1684x; 1.1684x over previous
"""Trainium2 Bass kernel for additive (Bahdanau) attention GNN message passing.

score[n, m] = v . tanh(a[n] + b[m]),  a = x1 @ W1.T, b = x2 @ W2.T + bc
w = softmax(score, axis=n);  ctx[m] = w[:, m].T @ x1
out = tanh(concat([att, ctx_s, ctx_e]) @ W_lin.T + b_lin)

Sharding: attender dim M=1024 split across 8 cores (128 each); attendees and
params replicated. No collectives.

Key trick: the per-(n,m,h) tanh (25M ACT elems/core in the naive scheme) is
replaced by a separable harmonic expansion
    tanh(s) ~ sum_k alpha_k sin(k w0 s),   s = a + b
    sin(k w0 (a+b)) = sin(k w0 a) cos(k w0 b) + cos(k w0 a) sin(k w0 b)
so the O(N*M*H) work becomes PE matmuls contracting h for each harmonic,
and the nonlinearity cost drops to O((N+M)*H) basis evaluations.

The HW ACT Sin table is only valid for |arg| < ~pi, so only small-argument
sins run on ACT (.5*w0*x, w0*x, 1.5*w0*x; args <= 2.7 rad) plus Squares;
cosines come from cos(2t) = 1 - 2 sin^2(t) and higher harmonics from
bf16 Chebyshev product recurrences on DVE. Weighting v*alpha folds into the
small b-side tiles (gpsimd). Scores accumulate in PSUM over 2F matmuls per
128-attendee chunk; softmax sums fall out of the ctx matmul via a ones
column in the attendee image; final linear runs in f32r.

PSUM accumulation note: start=True clears the has_written bits of the WHOLE
bank, so only the first matmul touching each bank may set it; later
first-writes to other regions rely on per-element overwrite-then-accumulate.
"""

import numpy as np
from ml_dtypes import bfloat16

import concourse.bass as bass
import concourse.tile as tile
from concourse import bacc, masks, mybir
from concourse.bass_utils import run_bass_kernel_spmd

F32 = mybir.dt.float32
F32R = mybir.dt.float32r
BF16 = mybir.dt.bfloat16
AF = mybir.ActivationFunctionType
OP = mybir.AluOpType

H = 128      # hidden
A = 256      # attention (output) size
N_S = 1024   # attendee statements
N_E = 512    # attendee EREs
M = 1024     # attenders
NC = 8       # cores
ML = M // NC # attenders per core
NT = N_S + N_E  # 1536
NCH = NT // 128  # 12 chunks of attendees
CW = 129     # x-image chunk width: 128 attendee cols + a ones column

W0 = 0.267059
KS = (1, 2, 3, 4, 6)
ALPHA = (1.17663, 0.12087, 0.17747, 0.13768, 0.13409)

_CACHE = {}


def _build():
    nc = bacc.Bacc(
        "TRN2", target_bir_lowering=False, debug=False, num_devices=NC
    )

    d_x16 = nc.dram_tensor("x16", [128, NCH * CW], BF16, kind="ExternalInput").ap()
    d_stmtsT = nc.dram_tensor("stmtsT", [128, N_S], BF16, kind="ExternalInput").ap()
    d_eresT = nc.dram_tensor("eresT", [128, N_E], BF16, kind="ExternalInput").ap()
    d_wa = nc.dram_tensor("wa", [128, 4 * H + ML + 4], BF16, kind="ExternalInput").ap()
    d_wlinT = nc.dram_tensor("wlinT", [128, 3 * A], BF16, kind="ExternalInput").ap()
    d_blin = nc.dram_tensor("blin", [1, A], BF16, kind="ExternalInput").ap()
    d_out = nc.dram_tensor("out", [ML, A], F32, kind="ExternalOutput").ap()

    with tile.TileContext(nc) as tc:
        _emit(nc, tc, d_x16, d_stmtsT, d_eresT, d_attT16, d_wT16,
              d_wlinT, d_vb, d_blin, d_out)

    nc.compile()
    return nc


def _emit(nc, tc, d_x16, d_stmtsT, d_eresT, d_attT16, d_wT16,
          d_wlinT, d_vb, d_blin, d_out):
    from contextlib import ExitStack

    ctx = ExitStack()
    with ctx:
        const = ctx.enter_context(tc.tile_pool(name="const", bufs=1))
        bpool = ctx.enter_context(tc.tile_pool(name="bpool", bufs=1))
        apool = ctx.enter_context(tc.tile_pool(name="apool", bufs=1))
        ps_a = ctx.enter_context(
            tc.tile_pool(name="ps_a", bufs=1, space=bass.MemorySpace.PSUM))
        ps_score = ctx.enter_context(
            tc.tile_pool(name="ps_score", bufs=1, space=bass.MemorySpace.PSUM))
        ps_small = ctx.enter_context(
            tc.tile_pool(name="ps_small", bufs=1, space=bass.MemorySpace.PSUM))

        # ---- gpsimd init + ACT table warm ----
        ident = const.tile([128, 128], F32)
        masks.make_identity(nc, ident[:])
        ones_row = const.tile([1, 128], BF16)
        nc.gpsimd.memset(ones_row[:], 1.0)
        scratch = const.tile([128, 1], F32)
        nc.gpsimd.memset(scratch[:], 0.0)
        nc.scalar.activation(scratch[:], scratch[:], AF.Sin)  # load trig table

        # ---- DMAs: front-critical on sync queue, epilogue-only on gpsimd ----
        sb_wa = const.tile([128, 4 * H + ML + 4], BF16)
        nc.sync.dma_start(sb_wa[:], d_wa[:, :])
        sb_wT = sb_wa[:, 0:4 * H]
        sb_attT16 = sb_wa[:, 4 * H:4 * H + ML]
        sb_vb = const.tile([128, 4], F32)
        nc.vector.tensor_copy(sb_vb[:], sb_wa[:, 4 * H + ML:4 * H + ML + 4])
        sb_stmtsT = const.tile([128, N_S], BF16)
        nc.sync.dma_start(sb_stmtsT[:], d_stmtsT[:, :])
        sb_eresT = const.tile([128, N_E], BF16)
        nc.sync.dma_start(sb_eresT[:], d_eresT[:, :])

        sb_x16 = const.tile([128, NCH * CW], BF16)
        nc.gpsimd.dma_start(sb_x16[:], d_x16[:, :])
        sb_wlinT = const.tile([128, 3 * A], BF16)
        nc.gpsimd.dma_start(sb_wlinT[:], d_wlinT[:, :])
        sb_blin = const.tile([1, A], BF16)
        nc.gpsimd.dma_start(sb_blin[0:1, :], d_blin[0:1, :])

        # ---- front matmuls ----
        # bT for both sets -> one [128, 258] PSUM tile (tag shared with ctx)
        ps_bT = ps_small.tile([128, 2 * CW], F32, tag="ctx", name="ps_bT")
        nc.tensor.matmul(ps_bT[:, 0:ML], sb_wT[:, 128:256], sb_attT16,
                         start=True, stop=True)
        nc.tensor.matmul(ps_bT[:, CW:CW + ML], sb_wT[:, 384:512], sb_attT16,
                         start=True, stop=True)
        sb_b2 = const.tile([128, 2 * ML], F32)
        nc.vector.tensor_scalar_add(sb_b2[:, 0:ML], ps_bT[:, 0:ML],
                                    sb_vb[:, 2:3])
        nc.vector.tensor_scalar_add(sb_b2[:, ML:2 * ML], ps_bT[:, CW:CW + ML],
                                    sb_vb[:, 3:4])

        # aT for both sets -> one [128, 1536] PSUM tile (bank-aligned pieces)
        ps_aT = ps_a.tile([128, NT], F32, tag="aT", name="ps_aT")
        nc.tensor.matmul(ps_aT[:, 0:512], sb_wT[:, 0:128],
                         sb_stmtsT[:, 0:512], start=True, stop=True)
        nc.tensor.matmul(ps_aT[:, 512:1024], sb_wT[:, 0:128],
                         sb_stmtsT[:, 512:1024], start=True, stop=True)
        nc.tensor.matmul(ps_aT[:, 1024:1536], sb_wT[:, 256:384],
                         sb_eresT[:], start=True, stop=True)

        # att + b_lin parts of the final linear (f32r, off critical path)
        ps_out = ps_small.tile([128, A], F32, tag="out")
        nc.tensor.matmul(ps_out[:], sb_attT16[:], sb_wlinT[:, 0:A],
                         start=True, stop=False, skip_group_check=True)
        nc.tensor.matmul(ps_out[:], ones_row[0:1, :], sb_blin[0:1, :],
                         start=False, stop=False, skip_group_check=True)

        # ---- basis generation helpers ----
        def gen_basis(pool, src_ap, width, name, act_square_s3):
            """Emit sin/cos harmonic tiles of src (bf16, [128, width]).

            Returns dict k -> (sin_tile, cos_tile)."""
            t = {}

            def tl(nm):
                return pool.tile([128, width], BF16, name=f"{nm}_{name}")

            u1, s1, u3 = tl("u1"), tl("s1"), tl("u3")
            nc.scalar.activation(u1[:], src_ap, AF.Sin, scale=0.5 * W0)
            nc.scalar.activation(s1[:], src_ap, AF.Sin, scale=W0)
            nc.scalar.activation(u3[:], src_ap, AF.Sin, scale=1.5 * W0)
            squ1, squ3 = tl("squ1"), tl("squ3")
            nc.scalar.activation(squ1[:], u1[:], AF.Square)
            nc.scalar.activation(squ3[:], u3[:], AF.Square)
            c1, c3 = tl("c1"), tl("c3")
            nc.vector.tensor_scalar(c1[:], squ1[:], -2.0, 1.0, OP.mult, OP.add)
            c1two = tl("c1two")
            nc.vector.tensor_scalar_mul(c1two[:], c1[:], 2.0)
            nc.vector.tensor_scalar(c3[:], squ3[:], -2.0, 1.0, OP.mult, OP.add)
            s2 = tl("s2")
            nc.vector.tensor_tensor(s2[:], c1two[:], s1[:], OP.mult)
            c2t, c2 = tl("c2t"), tl("c2")
            nc.vector.tensor_tensor(c2t[:], c1two[:], c1[:], OP.mult)
            nc.vector.tensor_scalar_sub(c2[:], c2t[:], 1.0)
            c2two = tl("c2two")
            nc.vector.tensor_scalar_mul(c2two[:], c2[:], 2.0)
            s3t, s3 = tl("s3t"), tl("s3")
            nc.vector.tensor_tensor(s3t[:], c1two[:], s2[:], OP.mult)
            nc.vector.tensor_tensor(s3[:], s3t[:], s1[:], OP.subtract)
            s4 = tl("s4")
            nc.vector.tensor_tensor(s4[:], c2two[:], s2[:], OP.mult)
            c4t, c4 = tl("c4t"), tl("c4")
            nc.vector.tensor_tensor(c4t[:], c2two[:], c2[:], OP.mult)
            nc.vector.tensor_scalar_sub(c4[:], c4t[:], 1.0)
            s6t, s6 = tl("s6t"), tl("s6")
            nc.vector.tensor_tensor(s6t[:], c2two[:], s4[:], OP.mult)
            nc.vector.tensor_tensor(s6[:], s6t[:], s2[:], OP.subtract)
            sqc3, c6 = tl("sqc3"), tl("c6")
            if act_square_s3:
                nc.scalar.activation(sqc3[:], c3[:], AF.Square)
            else:
                nc.vector.tensor_tensor(sqc3[:], c3[:], c3[:], OP.mult)
            nc.vector.tensor_scalar(c6[:], sqc3[:], 2.0, -1.0, OP.mult, OP.add)
            t[1] = (s1, c1)
            t[2] = (s2, c2)
            t[3] = (s3, c3)
            t[4] = (s4, c4)
            t[6] = (s6, c6)
            return t

        # b-side basis (small, both sets side by side) — DVE squares for s3
        bt = gen_basis(bpool, sb_b2[:], 2 * ML, "b", act_square_s3=False)

        # b-side weighting: w = bf16(v_set * alpha_k * tile_half), on gpsimd
        wsin = {}
        wcos = {}
        for k, al in zip(KS, ALPHA):
            ws = bpool.tile([128, 2 * ML], BF16, name=f"wsin{k}")
            wc = bpool.tile([128, 2 * ML], BF16, name=f"wcos{k}")
            for half, vcol in ((0, 0), (1, 1)):
                lo = half * ML
                nc.gpsimd.tensor_scalar(ws[:, lo:lo + ML],
                                        bt[k][0][:, lo:lo + ML],
                                        sb_vb[:, vcol:vcol + 1], al,
                                        OP.mult, OP.mult)
                nc.gpsimd.tensor_scalar(wc[:, lo:lo + ML],
                                        bt[k][1][:, lo:lo + ML],
                                        sb_vb[:, vcol:vcol + 1], al,
                                        OP.mult, OP.mult)
            wsin[k] = ws
            wcos[k] = wc

        # a-side basis (big tiles) — ACT square for s3 (balances engines)
        at = gen_basis(apool, ps_aT[:], NT, "a", act_square_s3=True)

        # ---- score matmuls: ps_sT[n_l, (c, m)] += sum_k sa*wcb + ca*wsb ----
        # start=True clears the has_written bits of the WHOLE PSUM bank, so
        # only the first matmul touching each bank (chunks 0/4/8) may set it;
        # later first-writes to other chunk regions rely on the per-element
        # overwrite-then-accumulate semantics.
        ps_sT = ps_score.tile([128, NT], F32)
        for ki, k in enumerate(KS):
            sa, ca = at[k]
            for trig in (0, 1):
                src = sa if trig == 0 else ca
                rhs = wcos[k] if trig == 0 else wsin[k]
                for c in range(NCH):
                    half = 0 if c < 8 else 1
                    lo = half * ML
                    nc.tensor.matmul(
                        ps_sT[:, c * 128:(c + 1) * 128],
                        src[:, c * 128:(c + 1) * 128],
                        rhs[:, lo:lo + ML],
                        start=(ki == 0 and trig == 0 and c % 4 == 0),
                        stop=(ki == len(KS) - 1 and trig == 1 and c % 4 == 3),
                        skip_group_check=True)

        # ---- epilogue: softmax + ctx + final linear + store ----
        sb_E = bpool.tile([128, NT], BF16)
        nc.scalar.activation(sb_E[:, 0:N_S], ps_sT[:, 0:N_S], AF.Exp)
        nc.scalar.activation(sb_E[:, N_S:NT], ps_sT[:, N_S:NT], AF.Exp)

        ps_ctx = ps_small.tile([128, 2 * CW], F32, tag="ctx", name="ps_ctx")
        for c in range(8):
            nc.tensor.matmul(ps_ctx[:, 0:CW],
                             sb_E[:, c * 128:(c + 1) * 128],
                             sb_x16[:, c * CW:(c + 1) * CW],
                             start=(c == 0), stop=(c == 7))
        for c in range(8, 12):
            nc.tensor.matmul(ps_ctx[:, CW:2 * CW],
                             sb_E[:, c * 128:(c + 1) * 128],
                             sb_x16[:, c * CW:(c + 1) * CW],
                             start=(c == 8), stop=(c == 11))

        sb_recip = bpool.tile([128, 2], F32)
        nc.vector.reciprocal(sb_recip[:, 0:1], ps_ctx[:, H:H + 1])
        nc.vector.reciprocal(sb_recip[:, 1:2], ps_ctx[:, CW + H:CW + H + 1])
        sb_ctx = bpool.tile([128, 2 * H], F32)
        nc.vector.tensor_scalar_mul(sb_ctx[:, 0:H], ps_ctx[:, 0:H],
                                    sb_recip[:, 0:1])
        nc.vector.tensor_scalar_mul(sb_ctx[:, H:2 * H], ps_ctx[:, CW:CW + H],
                                    sb_recip[:, 1:2])

        # transpose ctx halves [m, h] -> [h, m] via PE, copy to f32r
        ps_tr = ps_a.tile([128, NT], F32, tag="aT", name="ps_tr")
        nc.tensor.matmul(ps_tr[:, 0:128], sb_ctx[:, 0:H], ident[:],
                         is_transpose=True)
        nc.tensor.matmul(ps_tr[:, 512:640], sb_ctx[:, H:2 * H], ident[:],
                         is_transpose=True)
        sb_ctxT = bpool.tile([128, 2 * H], BF16)
        nc.vector.tensor_copy(sb_ctxT[:, 0:H], ps_tr[:, 0:128])
        nc.vector.tensor_copy(sb_ctxT[:, H:2 * H], ps_tr[:, 512:640])

        nc.tensor.matmul(ps_out[:], sb_ctxT[:, 0:H], sb_wlinT[:, A:2 * A],
                         start=False, stop=False, skip_group_check=True)
        nc.tensor.matmul(ps_out[:], sb_ctxT[:, H:2 * H], sb_wlinT[:, 2 * A:3 * A],
                         start=False, stop=True, skip_group_check=True)

        sb_out = bpool.tile([128, A], F32)
        nc.scalar.activation(sb_out[:], ps_out[:], AF.Tanh)
        nc.sync.dma_start(d_out[:, :], sb_out[:])


def _get_nc():
    if "nc" not in _CACHE:
        _CACHE["nc"] = _build()
    return _CACHE["nc"]


def _prep_inputs(inputs):
    """Host-side layout prep: transposes / bf16 casts / SBUF-image packing."""
    f = {k: np.ascontiguousarray(np.asarray(v, np.float32))
         for k, v in inputs.items()}
    stmts, eres = f["attendee_stmts"], f["attendee_eres"]
    ws, we, wlin = f["Ws_concat"], f["We_concat"], f["W_lin"]

    # x image: chunk c holds attendees [c*128, (c+1)*128) as [n_local, h],
    # plus a trailing ones column (turns the ctx matmul into ctx|sum)
    x = np.empty((128, NCH * CW), np.float32)
    for c in range(8):
        x[:, c * CW:c * CW + H] = stmts[c * 128:(c + 1) * 128]
        x[:, c * CW + H] = 1.0
    for c in range(8, 12):
        x[:, c * CW:c * CW + H] = eres[(c - 8) * 128:(c - 7) * 128]
        x[:, c * CW + H] = 1.0
    vb = np.stack([f["vs_single"], f["ve_single"], f["bs_concat"],
                   f["be_concat"]], axis=1)
    shared = {
        "x16": np.ascontiguousarray(x.astype(bfloat16)),
        "stmtsT": np.ascontiguousarray(stmts.T.astype(bfloat16)),
        "eresT": np.ascontiguousarray(eres.T.astype(bfloat16)),

        "wlinT": np.ascontiguousarray(np.concatenate(
            [wlin[:, 0:H].T, wlin[:, H:2 * H].T, wlin[:, 2 * H:3 * H].T],
            axis=1).astype(bfloat16)),
        "blin": np.ascontiguousarray(f["b_lin"][None, :].astype(bfloat16)),
    }
    wT16h = np.concatenate(
        [ws[:, :H].T, ws[:, H:].T, we[:, :H].T, we[:, H:].T], axis=1)
    att = f["attender"]
    in_maps = []
    for i in range(NC):
        attT = np.ascontiguousarray(att[i * ML:(i + 1) * ML].T)
        in_maps.append(dict(
            shared, attT16=np.ascontiguousarray(attT.astype(bfloat16))))
    return in_maps


def kernel(**inputs) -> np.ndarray:
    nc = _get_nc()
    in_maps = _prep_inputs(inputs)
    res = run_bass_kernel_spmd(nc, in_maps, list(range(NC)))
    return np.concatenate(
        [res.results[i]["out"].astype(np.float32) for i in range(NC)], axis=0)
